# revision 30
# baseline (speedup 1.0000x reference)
"""Trainium2 Bass kernel for nn_Model_20925080666713 (4-layer dense transformer).

Model (per reference): B=32, S=512, D=512, H=8, L=4, FFN=1024, fp32.
  out = x + pe
  per layer: Q,K,V = out@W* + b*; "raw view" attention over (B*H, S, DH)
  contiguous reshape; a = LN1(ctx@Wo + bo + out); out = LN2(relu(a@W1+b1)@W2 + b2 + a)

Sharding: pure data-parallel over batch across 8 NeuronCores (4 batch elems,
i.e. 2048 tokens, per core). Zero collectives. Weights replicated.

Key observation about the "faithful raw view": Q.reshape(B*H,S,DH) of the
contiguous (B,S,D) tensor makes attention BLOCK-LOCAL: slice (b,h) is the
contiguous 64-token x 512-channel block Q[b, 64h:64h+64, :] reinterpreted as
(512, 64) with row q = sm*8+dg (sm = s%64, dg = d//64) and col e = d%64.
So per 64-token block: att[q,kq] = sum_e Q[tb+sm, dg*64+e] K[tb+sm', dg'*64+e].

Device layout strategy (per core; QKV/W1/W2 bf16, V/Wo fp8 DoubleRow):
 - residual stream R token-major [128t x (16,512)], updated IN PLACE by LN2
 - PE-transposed copy feature-major [128d x (4,2048)] feeds projections
 - attention rows/cols use the "dg-major" order i = dg*64 + sm so all
   rearranges move contiguous 64-token runs:
   logits^T per block = 4 matmuls (K=64e, M=128 k-rows, N=512 q-rows) into
   2-bank [128,1024] PSUM tiles; one merged ACT Exp per pair
 - ctx^T = 4-step K=128 PSUM accumulation, lhsT = vh [128 k-rows, 65] with a
   ones column fusing the softmax denominators into PSUM row 64
 - normalize: K=1 broadcast matmul + DVE approx reciprocal + DVE multiply
 - V and Wo projections run as fp8e4m3 DoubleRow matmuls (weights x32 on
   host, descaled in the evac / LN); error cost measured ~+3e-3 only
 - LayerNorm: resid folded into PSUM by identity matmul; bn_stats on PSUM;
   rsqrt via bit-trick seed + 2 Newton steps (DVE, avoids ACT table swaps);
   fp16 evac tile; per-slice batched finalize; DVE apply
 - schedule: block loop software-pipelined (ctx[i-1] between logits[i]);
   the previous slice's FFN is interleaved into the block loop so the
   PE-heavy FFN overlaps the ACT-heavy softmax
 - all partition-crossing rearranges are SBUF->SBUF DMAs (block moves only)

The fast path assumes bv=bo=b2=0, ln*_g=1, ln*_b=0 (true for this problem's
setup_inputs); kernel() verifies at runtime and falls back to exact numpy
otherwise. bq, bk, b1 are applied on-device (free via ACT bias).
"""
import sys
if "/opt/trn_rl_repo" not in sys.path:
    sys.path.insert(0, "/opt/trn_rl_repo")

import numpy as np
import ml_dtypes

B, S, D, H, L, FFN = 32, 512, 512, 8, 4, 1024
DH = D // H
EPS = 1e-5
NCORES = 8
BL = B // NCORES          # batch per core
T = BL * S                # tokens per core = 2048
NCHUNK = T // 128         # 16 token chunks of 128
NSLICE = T // 512         # 4 token slices of 512
F32 = None  # set after imports
BF16 = None

_PROG_CACHE = {}


def _build_program(n_layers=L):
    import concourse.bass as bass
    import concourse.mybir as mybir
    import concourse.tile as tile
    from concourse import bacc
    from concourse.masks import make_identity

    f32 = mybir.dt.float32
    bf16 = mybir.dt.bfloat16
    f8 = mybir.dt.float8e4
    AF = mybir.ActivationFunctionType

    nc = bacc.Bacc("TRN2", target_bir_lowering=False, debug=False,
                   num_devices=NCORES)

    # ---- DRAM parameters (per-core shard of x / out; weights replicated) ----
    x_d = nc.dram_tensor("x", [BL, S, D], f32, kind="ExternalInput").ap()
    pe_d = nc.dram_tensor("pe", [S, D], f32, kind="ExternalInput").ap()
    wq_d = nc.dram_tensor("wq", [L, D, D], bf16, kind="ExternalInput").ap()
    wk_d = nc.dram_tensor("wk", [L, D, D], bf16, kind="ExternalInput").ap()
    wv_d = nc.dram_tensor("wv", [L, D, D], f8, kind="ExternalInput").ap()
    wo_d = nc.dram_tensor("wo", [L, D, D], f8, kind="ExternalInput").ap()
    w1_d = nc.dram_tensor("w1", [L, D, FFN], bf16, kind="ExternalInput").ap()
    w2_d = nc.dram_tensor("w2", [L, FFN, D], bf16, kind="ExternalInput").ap()
    bq_d = nc.dram_tensor("bq", [L, D], f32, kind="ExternalInput").ap()
    bk_d = nc.dram_tensor("bk", [L, D], f32, kind="ExternalInput").ap()
    b1_d = nc.dram_tensor("b1", [L, FFN], f32, kind="ExternalInput").ap()
    out_d = nc.dram_tensor("out", [BL, S * D], f32, kind="ExternalOutput").ap()
    ov = out_d.rearrange("b (s d) -> b s d", d=D)

    with tile.TileContext(nc) as tc:
        _emit(nc, tc, tile, mybir, make_identity, n_layers,
              x_d, pe_d, wq_d, wk_d, wv_d, wo_d, w1_d, w2_d,
              bq_d, bk_d, b1_d, ov)
    nc.finalize()
    return nc


def _emit(nc, tc, tile, mybir, make_identity, n_layers,
          x_d, pe_d, wq_d, wk_d, wv_d, wo_d, w1_d, w2_d, bq_d, bk_d, b1_d, ov):
    from contextlib import ExitStack
    import concourse.bass as bass

    f32 = mybir.dt.float32
    bf16 = mybir.dt.bfloat16
    f16 = mybir.dt.float16
    f8 = mybir.dt.float8e4
    PM = mybir.MatmulPerfMode
    AF = mybir.ActivationFunctionType
    OP = mybir.AluOpType

    ctx = ExitStack()
    with ctx:
        # ---------------- pools ----------------
        consts = ctx.enter_context(tc.tile_pool(name="consts", bufs=1))
        stream = ctx.enter_context(tc.tile_pool(name="stream", bufs=2))
        streamT = ctx.enter_context(tc.tile_pool(name="streamT", bufs=2))
        wq_p = ctx.enter_context(tc.tile_pool(name="wq_p", bufs=1))
        wk_p = ctx.enter_context(tc.tile_pool(name="wk_p", bufs=1))
        wv_p = ctx.enter_context(tc.tile_pool(name="wv_p", bufs=1))
        wo_p = ctx.enter_context(tc.tile_pool(name="wo_p", bufs=1))
        w1_p = ctx.enter_context(tc.tile_pool(name="w1_p", bufs=1))
        w2_p = ctx.enter_context(tc.tile_pool(name="w2_p", bufs=1))
        qt_p = ctx.enter_context(tc.tile_pool(name="qt_p", bufs=2))
        kt_p = ctx.enter_context(tc.tile_pool(name="kt_p", bufs=2))
        vtok_p = ctx.enter_context(tc.tile_pool(name="vtok_p", bufs=2))
        qhT_p = ctx.enter_context(tc.tile_pool(name="qhT_p", bufs=2))
        kd_p = ctx.enter_context(tc.tile_pool(name="kd_p", bufs=2))
        vh_p = ctx.enter_context(tc.tile_pool(name="vh_p", bufs=2))
        recip_p = ctx.enter_context(tc.tile_pool(name="recip_p", bufs=2))
        attexp_p = ctx.enter_context(tc.tile_pool(name="attexp_p", bufs=3))
        ctxsb_p = ctx.enter_context(tc.tile_pool(name="ctxsb_p", bufs=3))
        ctxt_p = ctx.enter_context(tc.tile_pool(name="ctxt_p", bufs=5))
        ht_p = ctx.enter_context(tc.tile_pool(name="ht_p", bufs=1))
        rt8_p = ctx.enter_context(tc.tile_pool(name="rt8_p", bufs=2))
        lnin_p = ctx.enter_context(tc.tile_pool(name="lnin_p", bufs=2))
        lntmp_p = ctx.enter_context(tc.tile_pool(name="lntmp_p", bufs=5))
        stats_p = ctx.enter_context(tc.tile_pool(name="stats_p", bufs=4))
        ps_p = ctx.enter_context(tc.tile_pool(name="ps_p", bufs=2, space="PSUM"))
        attps_p = ctx.enter_context(tc.tile_pool(name="attps_p", bufs=2, space="PSUM"))
        bcps_p = ctx.enter_context(tc.tile_pool(name="bcps_p", bufs=1, space="PSUM"))
        ctxps_p = ctx.enter_context(tc.tile_pool(name="ctxps_p", bufs=1, space="PSUM"))

        # ---------------- constants ----------------
        ident = consts.tile([128, 128], bf16, tag="ident")
        make_identity(nc, ident)
        ident32 = consts.tile([128, 128], bf16, tag="ident32")
        nc.vector.tensor_scalar(out=ident32, in0=ident, scalar1=32.0,
                                scalar2=None, op0=OP.mult)
        pe_sb = consts.tile([128, 4, D], bf16, tag="pe_sb")
        for sc in range(4):
            pe_st = lnin_p.tile([128, 512], f32, tag="lnin", name=f"pe_st{sc}")
            nc.sync.dma_start(out=pe_st, in_=pe_d[sc * 128:sc * 128 + 128, :])
            nc.vector.tensor_copy(pe_sb[:, sc, :], pe_st)
        bq_sb = consts.tile([128, L, 4], f32, tag="bq_sb")
        nc.sync.dma_start(out=bq_sb, in_=bq_d.rearrange("l (a p) -> p l a", p=128))
        bk_sb = consts.tile([128, L, 4], f32, tag="bk_sb")
        nc.sync.dma_start(out=bk_sb, in_=bk_d.rearrange("l (a p) -> p l a", p=128))
        b1_sb = consts.tile([128, L, 8], f32, tag="b1_sb")
        nc.sync.dma_start(out=b1_sb, in_=b1_d.rearrange("l (a p) -> p l a", p=128))
        ones_r = consts.tile([128, 128], bf16, tag="ones_r")
        nc.vector.memset(ones_r, 1.0)

        def transpose_stream(src, dt=bf16):
            """token-major [128,(16),512] -> new feature-major [128,(4),2048]."""
            dst = streamT.tile([128, 4, T], dt, tag="streamT")
            for dj in range(4):
                for tg in range(NCHUNK // 4):
                    ps = ps_p.tile([128, 512], bf16, tag="ps")
                    for k in range(4):
                        tcn = tg * 4 + k
                        nc.tensor.transpose(
                            ps[:, k * 128:(k + 1) * 128],
                            src[:, tcn, dj * 128:(dj + 1) * 128], ident)
                    nc.scalar.activation(dst[:, dj, tg * 512:(tg + 1) * 512], ps,
                                         AF.Copy)
            return dst

        i32 = mybir.dt.int32

        def ln_pre(ps, mvs, tcw):
            """LN stats of PSUM chunk (resid already folded in by an identity
            matmul) + ACT evacuation to bf16; apply happens per slice."""
            st6 = stats_p.tile([128, 6], f32, tag="st6")
            nc.vector.bn_stats(st6, ps)
            nc.vector.bn_aggr(mvs[:, tcw, :], st6)
            tmp = lntmp_p.tile([128, 512], f16, tag="lntmp")
            nc.scalar.activation(tmp, ps, AF.Copy)
            return tmp

        def ln_finalize(mvs):
            """Batched rstd = rsqrt(var+eps) via bit-trick seed + 2 Newton
            iterations (all DVE; avoids ACT Sqrt = table reload vs Exp),
            and nm = -mean*rstd so the apply is (x*rstd + nm)."""
            u = stats_p.tile([128, 4], f32, tag="u4")
            nc.vector.tensor_scalar(out=u, in0=mvs[:, :, 1], scalar1=EPS,
                                    scalar2=None, op0=OP.add)
            y = stats_p.tile([128, 4], f32, tag="y4")
            nc.vector.tensor_scalar(out=y.bitcast(i32), in0=u.bitcast(i32),
                                    scalar1=1, scalar2=-1,
                                    op0=OP.logical_shift_right,
                                    op1=OP.bitwise_xor)
            nc.vector.tensor_scalar(out=y.bitcast(i32), in0=y.bitcast(i32),
                                    scalar1=0x5f3759e0, scalar2=None, op0=OP.add)
            a = stats_p.tile([128, 4], f32, tag="a4")
            for _ in range(2):
                nc.vector.tensor_mul(a, u, y)
                nc.vector.tensor_mul(a, a, y)
                nc.vector.tensor_scalar(out=a, in0=a, scalar1=-0.5, scalar2=1.5,
                                        op0=OP.mult, op1=OP.add)
                nc.vector.tensor_mul(y, y, a)
            nm = stats_p.tile([128, 4], f32, tag="nm4")
            nc.vector.scalar_tensor_tensor(out=nm, in0=mvs[:, :, 0], scalar=-1.0,
                                           in1=y, op0=OP.mult, op1=OP.mult)
            return y, nm

        def ln_apply(tmp, y, nm, tcw, out_ap):
            nc.vector.tensor_scalar(out=out_ap, in0=tmp,
                                    scalar1=y[:, tcw:tcw + 1],
                                    scalar2=nm[:, tcw:tcw + 1],
                                    op0=OP.mult, op1=OP.add)

        # ---------------- prologue: R0 = x + pe ----------------
        # R is THE residual stream, updated in place by LN2 applies (a
        # chunk's last reader is the Wo identity matmul of its own slice)
        R = stream.tile([128, NCHUNK, 512], bf16, tag="resid", bufs=1)
        for tcn in range(NCHUNK):
            xt = lnin_p.tile([128, 512], f32, tag="lnin")
            nc.sync.dma_start(out=xt, in_=x_d[tcn // 4,
                                             (tcn % 4) * 128:(tcn % 4) * 128 + 128, :])
            nc.vector.tensor_add(R[:, tcn, :], xt, pe_sb[:, tcn % 4, :])

        # ---------------- layers ----------------
        for l in range(n_layers):
            # -- weights for this layer --
            wq_t = wq_p.tile([128, 4, D], bf16, tag="wq")
            wk_t = wk_p.tile([128, 4, D], bf16, tag="wk")
            wv_t = wv_p.tile([128, 4, D], f8, tag="wv")
            wo_t = wo_p.tile([128, 4, D], f8, tag="wo")
            w1_t = w1_p.tile([128, 4, FFN], bf16, tag="w1")
            w2_t = w2_p.tile([128, 8, D], bf16, tag="w2")
            for w_t, w_d, nk in ((wq_t, wq_d, 4), (wk_t, wk_d, 4),
                                 (wv_t, wv_d, 4), (wo_t, wo_d, 4),
                                 (w1_t, w1_d, 4), (w2_t, w2_d, 8)):
                nc.sync.dma_start(
                    out=w_t, in_=w_d[l].rearrange("(k p) n -> p k n", p=128))

            rt = transpose_stream(R)  # feature-major stream
            A = stream.tile([128, NCHUNK, 512], bf16, tag="stream", bufs=1)
            at = streamT.tile([128, 4, T], bf16, tag="streamT")
            last = l == n_layers - 1
            ffn_state = {}

            def ffn_chunk(fs, step, at=at, A=A, w1_t=w1_t, w2_t=w2_t,
                          l=l, last=last, ffn_state=ffn_state):
                """FFN for slice fs, emitted interleaved into the next
                slice's attention block loop (layer state bound at def time
                so the carried-over closure uses ITS layer's tiles).
                steps 0-3: W1 fc-pair + relu; steps 4-7: W2 token-chunk +
                ln_pre (finalize+apply at 7)."""
                t0f = fs * 512
                if step == 0:
                    ffn_state["ht"] = ht_p.tile([128, 8, 512], bf16, tag="ht", name=f"ht{fs}")
                    ffn_state["mvs"] = stats_p.tile([128, 4, 2], f32, tag="mvs", name=f"mvsf{fs}")
                    ffn_state["tmps"] = []
                ht_sl = ffn_state["ht"]
                if step < 4:
                    for fc in (2 * step, 2 * step + 1):
                        ps = ps_p.tile([128, 512], f32, tag="ps")
                        for dk in range(4):
                            nc.tensor.matmul(ps, w1_t[:, dk, fc * 128:fc * 128 + 128],
                                             at[:, dk, t0f:t0f + 512],
                                             start=(dk == 0), stop=(dk == 3))
                        nc.scalar.activation(ht_sl[:, fc, :], ps, AF.Relu,
                                             bias=b1_sb[:, l, fc:fc + 1].opt())
                    return
                tcw = step - 4
                tcn = fs * 4 + tcw
                ps = ps_p.tile([128, 512], f32, tag="ps")
                nc.tensor.matmul(ps, ident, A[:, tcn, :], start=True, stop=False)
                for fk in range(8):
                    nc.tensor.matmul(ps, ht_sl[:, fk, tcw * 128:tcw * 128 + 128],
                                     w2_t[:, fk, :], start=False, stop=(fk == 7))
                ffn_state["tmps"].append(ln_pre(ps, ffn_state["mvs"], tcw))
                if step == 7:
                    y2, nm2 = ln_finalize(ffn_state["mvs"])
                    for tc2 in range(4):
                        tcn2 = fs * 4 + tc2
                        if last:
                            ot = lnin_p.tile([128, 512], f32, tag="lnin")
                            ln_apply(ffn_state["tmps"][tc2], y2, nm2, tc2, ot)
                            nc.sync.dma_start(
                                out=ov[tcn2 // 4,
                                       (tcn2 % 4) * 128:(tcn2 % 4) * 128 + 128, :],
                                in_=ot)
                        else:
                            ln_apply(ffn_state["tmps"][tc2], y2, nm2, tc2,
                                     R[:, tcn2, :])


            def emit_qkv(ts):
                """Q/K/V projections + rearrange DMAs for one slice.

                Layouts (i = dg*64 + sm "dg-major" row order everywhere):
                  qd [64e, blk, 512 j]  j = dg*64+sm (att columns = q-rows)
                  kd [64e, mc, blk, 128]  lhsT panel for chunk mc covers
                     k-rows i in [128mc, 128mc+128) as (par, sm)
                  vh [128p, blk, mc, 65]  p = par*64+sm' -> k-row i=128mc+p,
                     free = 64 V feats of group dg'=2mc+par + ones column
                """
                t0 = ts * 512
                qt_t = qt_p.tile([128, 4, 512], bf16, tag="qt", name=f"qt{ts}")
                kt_t = kt_p.tile([128, 4, 512], bf16, tag="kt", name=f"kt{ts}")
                for (w_t, b_sb, dst) in ((wq_t, bq_sb, qt_t), (wk_t, bk_sb, kt_t)):
                    for dc in range(4):
                        ps = ps_p.tile([128, 512], f32, tag="ps", name=f"ps{ts}{dc}")
                        for dk in range(4):
                            nc.tensor.matmul(ps, w_t[:, dk, dc * 128:dc * 128 + 128],
                                             rt[:, dk, t0:t0 + 512],
                                             start=(dk == 0), stop=(dk == 3))
                        nc.vector.tensor_scalar(
                            out=dst[:, dc, :], in0=ps,
                            scalar1=b_sb[:, l, dc:dc + 1].opt(), scalar2=None,
                            op0=OP.add)
                kd_t = kd_p.tile([64, 4, 8, 128], bf16, tag="kd", name=f"kd{ts}")
                kd_v = kd_t.rearrange("p m b (r s) -> p m b r s", r=2)
                kt_v = kt_t.rearrange("p m (b s) -> p m b s", b=8)
                nc.sync.dma_start(out=kd_v[0:64, :, :, 0, :],
                                  in_=kt_v[0:64, :, :, :])
                nc.sync.dma_start(out=kd_v[0:64, :, :, 1, :],
                                  in_=kt_v[64:128, :, :, :])
                qd_sl = qhT_p.tile([64, 8, 512], bf16, tag="qhT", name=f"qd{ts}")
                qd_v = qd_sl.rearrange("p b (a c) -> p b a c", a=4)
                qt_v = qt_t.rearrange("p a (b c) -> p b a c", b=8)
                for dt4 in range(4):
                    nc.sync.dma_start(out=qd_v[0:64, :, dt4, 0:64],
                                      in_=qt_v[0:64, :, dt4, :])
                    nc.sync.dma_start(out=qd_v[0:64, :, dt4, 64:128],
                                      in_=qt_v[64:128, :, dt4, :])
                rt8_sl = rt8_p.tile([128, 4, 512], f8, tag="rt8", name=f"rt8{ts}")
                for dk in range(4):
                    nc.scalar.copy(rt8_sl[:, dk, :], rt[:, dk, t0:t0 + 512])
                vtok_sl = vtok_p.tile([128, 4, 512], bf16, tag="vtok", name=f"vt{ts}")
                for tcw in range(4):
                    ps = ps_p.tile([128, 512], f32, tag="ps", name=f"psv{ts}{tcw}")
                    for st2 in range(2):
                        nc.tensor.matmul(
                            ps,
                            rt8_sl[:, 2 * st2:2 * st2 + 2, tcw * 128:tcw * 128 + 128],
                            wv_t[:, 2 * st2:2 * st2 + 2, :],
                            start=(st2 == 0), stop=(st2 == 1), perf_mode=PM.DoubleRow)
                    nc.vector.tensor_scalar(out=vtok_sl[:, tcw, :], in0=ps,
                                            scalar1=1.0 / 32.0, scalar2=None,
                                            op0=OP.mult)
                vh_t = vh_p.tile([128, 8, 4, 65], bf16, tag="vh", name=f"vh{ts}")
                nc.vector.memset(vh_t[:, :, :, 64:65], 1.0)
                for mc in range(4):
                    for par in range(2):
                        f0 = (2 * mc + par) * 64
                        nc.sync.dma_start(
                            out=vh_t[par * 64:par * 64 + 64, 0::2, mc, 0:64],
                            in_=vtok_sl[0:64, :, f0:f0 + 64])
                        nc.sync.dma_start(
                            out=vh_t[par * 64:par * 64 + 64, 1::2, mc, 0:64],
                            in_=vtok_sl[64:128, :, f0:f0 + 64])
                return qd_sl, kd_t, vh_t

            slice_ops = emit_qkv(0)
            for ts in range(NSLICE):
                qd_sl, kd_t, vh_t = slice_ops

                # -- attention: 8 blocks of 64 tokens, software-pipelined
                # (ctx of block i-1 between logits of block i), with the
                # previous slice's FFN chunks interleaved so the PE-heavy
                # FFN overlaps the ACT-heavy softmax --
                ctx_ch = []
                axs_prev, prev_blk, ctxc = None, None, None
                for step in range(9):
                    blk = step if step < 8 else None
                    axs = []
                    if blk is not None:
                        for mp in range(2):
                            aps = attps_p.tile([128, 1024], f32, tag="attps")
                            for mh in range(2):
                                nc.tensor.matmul(
                                    aps[:, mh * 512:mh * 512 + 512],
                                    kd_t[0:64, 2 * mp + mh, blk, :],
                                    qd_sl[0:64, blk, :], start=True, stop=True)
                            ax = attexp_p.tile([128, 1024], bf16, tag="attexp")
                            nc.scalar.activation(ax, aps, AF.Exp,
                                                 scale=float(DH ** -0.5))
                            axs.append(ax)
                    if prev_blk is not None:
                        pb = prev_blk
                        half = pb % 2
                        cps = ctxps_p.tile([65, 512], f32, tag="ctxps")
                        for mc in range(4):
                            nc.tensor.matmul(
                                cps, vh_t[:, pb, mc, :],
                                axs_prev[mc // 2][:, (mc % 2) * 512:(mc % 2) * 512 + 512],
                                start=(mc == 0), stop=(mc == 3))
                        csb = ctxsb_p.tile([65, 512], bf16, tag="ctxsb")
                        nc.vector.tensor_copy(csb, cps)
                        bcp = bcps_p.tile([64, 512], f32, tag="bcp")
                        nc.tensor.matmul(bcp, ones_r[64:65, 0:64],
                                         csb[64:65, :], start=True, stop=True)
                        rcf = recip_p.tile([64, 512], f32, tag="recip")
                        nc.vector.reciprocal_approx_fast(out=rcf, in_=bcp)
                        nc.vector.tensor_mul(csb[0:64, :], csb[0:64, :], rcf)
                        csb_v = csb.rearrange("p (a c) -> p a c", a=4)
                        if half == 0:
                            ctxc = ctxt_p.tile([128, 4, 128], bf16, tag="ctxt")
                        c0 = half * 64
                        nc.sync.dma_start(out=ctxc[0:64, :, c0:c0 + 64],
                                          in_=csb_v[0:64, :, 0:64])
                        nc.sync.dma_start(out=ctxc[64:128, :, c0:c0 + 64],
                                          in_=csb_v[0:64, :, 64:128])
                        if half == 1:
                            ctxc8 = ctxt_p.tile([128, 4, 128], f8, tag="ctxt8")
                            nc.scalar.copy(ctxc8, ctxc)
                            ctx_ch.append(ctxc8)
                    axs_prev, prev_blk = axs, blk

                    if ts > 0 and step < 8:
                        ffn_chunk(ts - 1, step)
                    if step == 3 and ts + 1 < NSLICE:
                        slice_ops = emit_qkv(ts + 1)

                # -- Wo projection + residual (identity matmul) + LN1 --
                mvs1 = stats_p.tile([128, 4, 2], f32, tag="mvs")
                tmps1 = []
                for tcw in range(4):
                    tcn = ts * 4 + tcw
                    ps = ps_p.tile([128, 512], f32, tag="ps")
                    nc.tensor.matmul(ps, ident32, R[:, tcn, :],
                                     start=True, stop=False)
                    for st2 in range(2):
                        nc.tensor.matmul(
                            ps, ctx_ch[tcw][:, 2 * st2:2 * st2 + 2, :],
                            wo_t[:, 2 * st2:2 * st2 + 2, :],
                            start=False, stop=(st2 == 1), perf_mode=PM.DoubleRow)
                    tmps1.append(ln_pre(ps, mvs1, tcw))
                y1, nm1 = ln_finalize(mvs1)
                for tcw in range(4):
                    ln_apply(tmps1[tcw], y1, nm1, tcw, A[:, ts * 4 + tcw, :])

                # -- feature-major transpose of this slice of A --
                for dj in range(4):
                    ps = ps_p.tile([128, 512], bf16, tag="ps")
                    for k in range(4):
                        tcn = ts * 4 + k
                        nc.tensor.transpose(
                            ps[:, k * 128:(k + 1) * 128],
                            A[:, tcn, dj * 128:(dj + 1) * 128], ident)
                    nc.scalar.activation(at[:, dj, ts * 512:(ts + 1) * 512], ps,
                                         AF.Copy)

            # trailing FFN for the last slice
            for step in range(8):
                ffn_chunk(NSLICE - 1, step)


# ---------------------------------------------------------------------------
# host side
# ---------------------------------------------------------------------------

def _numpy_reference(x, pe, Wq, bq, Wk, bk, Wv, bv, Wo, bo, ln1_g, ln1_b,
                     W1, b1, W2, b2, ln2_g, ln2_b):
    """Exact fp64->fp32 fallback, mirrors reference.py (used only if the
    fast-path constant assumptions do not hold)."""
    def ln(x_, g, b_):
        mu = x_.mean(-1, keepdims=True)
        var = ((x_ - mu) ** 2).mean(-1, keepdims=True)
        return (x_ - mu) / np.sqrt(var + EPS) * g + b_
    out = x.astype(np.float64) + pe.astype(np.float64)
    scale = DH ** -0.5
    for l in range(L):
        Q = out @ Wq[l].astype(np.float64) + bq[l]
        K = out @ Wk[l].astype(np.float64) + bk[l]
        V = out @ Wv[l].astype(np.float64) + bv[l]
        Qh = Q.reshape(B * H, S, DH)
        Kh = K.reshape(B * H, S, DH)
        Vh = V.reshape(B * H, S, DH)
        att = np.einsum("bqd,bkd->bqk", Qh, Kh) * scale
        att = att - att.max(-1, keepdims=True)
        att = np.exp(att)
        att /= att.sum(-1, keepdims=True)
        ctxv = np.einsum("bqk,bkd->bqd", att, Vh).reshape(B, S, D)
        a = ln(ctxv @ Wo[l].astype(np.float64) + bo[l] + out, ln1_g[l], ln1_b[l])
        h = np.maximum(a @ W1[l].astype(np.float64) + b1[l], 0.0)
        out = ln(h @ W2[l].astype(np.float64) + b2[l] + a, ln2_g[l], ln2_b[l])
    return out.reshape(B, S * D).astype(np.float32)


def _fast_path_ok(inputs):
    z = lambda a: np.all(np.asarray(a) == 0.0)
    o = lambda a: np.all(np.asarray(a) == 1.0)
    return (z(inputs["bv"]) and z(inputs["bo"]) and z(inputs["b2"])
            and o(inputs["ln1_g"]) and z(inputs["ln1_b"])
            and o(inputs["ln2_g"]) and z(inputs["ln2_b"]))


def kernel(**inputs):
    inputs = {k: np.asarray(v) for k, v in inputs.items()}
    if not _fast_path_ok(inputs):
        return _numpy_reference(**inputs)

    res = _run(inputs)
    return np.concatenate([res.results[i]["out"] for i in range(NCORES)], axis=0)


def _run(inputs, trace=False, **kw):
    from concourse.bass_utils import run_bass_kernel_spmd

    if "prog" not in _PROG_CACHE:
        _PROG_CACHE["prog"] = _build_program(L)
    nc = _PROG_CACHE["prog"]

    bf = ml_dtypes.bfloat16
    f8 = ml_dtypes.float8_e4m3
    shared = {
        "pe": inputs["pe"].astype(np.float32),
        "wq": inputs["Wq"].astype(bf), "wk": inputs["Wk"].astype(bf),
        "wv": (inputs["Wv"] * 32.0).astype(f8),
        "wo": (inputs["Wo"] * 32.0).astype(f8),
        "w1": inputs["W1"].astype(bf), "w2": inputs["W2"].astype(bf),
        "bq": inputs["bq"].astype(np.float32),
        "bk": inputs["bk"].astype(np.float32),
        "b1": inputs["b1"].astype(np.float32),
    }
    x = inputs["x"].astype(np.float32)
    in_maps = [dict(shared, x=np.ascontiguousarray(x[i * BL:(i + 1) * BL]))
               for i in range(NCORES)]
    return run_bass_kernel_spmd(nc, in_maps, list(range(NCORES)),
                                trace=trace, **kw)


if __name__ == "__main__":
    import reference
    ins = {k: np.asarray(v) for k, v in reference.setup_inputs().items()}
    got = kernel(**ins)
    print("out shape:", got.shape, got.dtype)



# revision 31
# speedup vs baseline: 1.0396x; 1.0396x over previous
"""Trainium2 Bass kernel for nn_Model_20925080666713 (4-layer dense transformer).

Model (per reference): B=32, S=512, D=512, H=8, L=4, FFN=1024, fp32.
  out = x + pe
  per layer: Q,K,V = out@W* + b*; "raw view" attention over (B*H, S, DH)
  contiguous reshape; a = LN1(ctx@Wo + bo + out); out = LN2(relu(a@W1+b1)@W2 + b2 + a)

Sharding: pure data-parallel over batch across 8 NeuronCores (4 batch elems,
i.e. 2048 tokens, per core). Zero collectives. Weights replicated.

Key observation about the "faithful raw view": Q.reshape(B*H,S,DH) of the
contiguous (B,S,D) tensor makes attention BLOCK-LOCAL: slice (b,h) is the
contiguous 64-token x 512-channel block Q[b, 64h:64h+64, :] reinterpreted as
(512, 64) with row q = sm*8+dg (sm = s%64, dg = d//64) and col e = d%64.
So per 64-token block: att[q,kq] = sum_e Q[tb+sm, dg*64+e] K[tb+sm', dg'*64+e].

Device layout strategy (per core; QKV/W1/W2 bf16, V/Wo fp8 DoubleRow):
 - residual stream R token-major [128t x (16,512)], updated IN PLACE by LN2
 - PE-transposed copy feature-major [128d x (4,2048)] feeds projections
 - attention rows/cols use the "dg-major" order i = dg*64 + sm so all
   rearranges move contiguous 64-token runs:
   logits^T per block = 4 matmuls (K=64e, M=128 k-rows, N=512 q-rows) into
   2-bank [128,1024] PSUM tiles; one merged ACT Exp per pair
 - ctx^T = 4-step K=128 PSUM accumulation, lhsT = vh [128 k-rows, 65] with a
   ones column fusing the softmax denominators into PSUM row 64
 - normalize: K=1 broadcast matmul + DVE approx reciprocal + DVE multiply
 - V and Wo projections run as fp8e4m3 DoubleRow matmuls (weights x32 on
   host, descaled in the evac / LN); error cost measured ~+3e-3 only
 - LayerNorm: resid folded into PSUM by identity matmul; bn_stats on PSUM;
   rsqrt via bit-trick seed + 2 Newton steps (DVE, avoids ACT table swaps);
   fp16 evac tile; per-slice batched finalize; DVE apply
 - schedule: block loop software-pipelined (ctx[i-1] between logits[i]);
   the previous slice's FFN is interleaved into the block loop so the
   PE-heavy FFN overlaps the ACT-heavy softmax
 - all partition-crossing rearranges are SBUF->SBUF DMAs (block moves only)

The fast path assumes bv=bo=b2=0, ln*_g=1, ln*_b=0 (true for this problem's
setup_inputs); kernel() verifies at runtime and falls back to exact numpy
otherwise. bq, bk, b1 are applied on-device (free via ACT bias).
"""
import sys
if "/opt/trn_rl_repo" not in sys.path:
    sys.path.insert(0, "/opt/trn_rl_repo")

import numpy as np
import ml_dtypes

B, S, D, H, L, FFN = 32, 512, 512, 8, 4, 1024
DH = D // H
EPS = 1e-5
NCORES = 8
BL = B // NCORES          # batch per core
T = BL * S                # tokens per core = 2048
NCHUNK = T // 128         # 16 token chunks of 128
NSLICE = T // 512         # 4 token slices of 512
F32 = None  # set after imports
BF16 = None

_PROG_CACHE = {}


def _build_program(n_layers=L):
    import concourse.bass as bass
    import concourse.mybir as mybir
    import concourse.tile as tile
    from concourse import bacc
    from concourse.masks import make_identity

    f32 = mybir.dt.float32
    bf16 = mybir.dt.bfloat16
    f8 = mybir.dt.float8e4
    AF = mybir.ActivationFunctionType

    nc = bacc.Bacc("TRN2", target_bir_lowering=False, debug=False,
                   num_devices=NCORES)

    # ---- DRAM parameters (per-core shard of x / out; weights replicated) ----
    x_d = nc.dram_tensor("x", [BL, S, D], f32, kind="ExternalInput").ap()
    pe_d = nc.dram_tensor("pe", [S, D], f32, kind="ExternalInput").ap()
    wq_d = nc.dram_tensor("wq", [L, D, D], bf16, kind="ExternalInput").ap()
    wk_d = nc.dram_tensor("wk", [L, D, D], bf16, kind="ExternalInput").ap()
    wv_d = nc.dram_tensor("wv", [L, D, D], f8, kind="ExternalInput").ap()
    wo_d = nc.dram_tensor("wo", [L, D, D], f8, kind="ExternalInput").ap()
    w1_d = nc.dram_tensor("w1", [L, D, FFN], bf16, kind="ExternalInput").ap()
    w2_d = nc.dram_tensor("w2", [L, FFN, D], bf16, kind="ExternalInput").ap()
    bq_d = nc.dram_tensor("bq", [L, D], f32, kind="ExternalInput").ap()
    bk_d = nc.dram_tensor("bk", [L, D], f32, kind="ExternalInput").ap()
    b1_d = nc.dram_tensor("b1", [L, FFN], f32, kind="ExternalInput").ap()
    out_d = nc.dram_tensor("out", [BL, S * D], f32, kind="ExternalOutput").ap()
    ov = out_d.rearrange("b (s d) -> b s d", d=D)

    with tile.TileContext(nc) as tc:
        _emit(nc, tc, tile, mybir, make_identity, n_layers,
              x_d, pe_d, wq_d, wk_d, wv_d, wo_d, w1_d, w2_d,
              bq_d, bk_d, b1_d, ov)
    nc.finalize()
    return nc


def _emit(nc, tc, tile, mybir, make_identity, n_layers,
          x_d, pe_d, wq_d, wk_d, wv_d, wo_d, w1_d, w2_d, bq_d, bk_d, b1_d, ov):
    from contextlib import ExitStack
    import concourse.bass as bass

    f32 = mybir.dt.float32
    bf16 = mybir.dt.bfloat16
    f16 = mybir.dt.float16
    f8 = mybir.dt.float8e4
    PM = mybir.MatmulPerfMode
    AF = mybir.ActivationFunctionType
    OP = mybir.AluOpType

    ctx = ExitStack()
    with ctx:
        # ---------------- pools ----------------
        consts = ctx.enter_context(tc.tile_pool(name="consts", bufs=1))
        stream = ctx.enter_context(tc.tile_pool(name="stream", bufs=2))
        streamT = ctx.enter_context(tc.tile_pool(name="streamT", bufs=2))
        wq_p = ctx.enter_context(tc.tile_pool(name="wq_p", bufs=1))
        wk_p = ctx.enter_context(tc.tile_pool(name="wk_p", bufs=1))
        wv_p = ctx.enter_context(tc.tile_pool(name="wv_p", bufs=1))
        wo_p = ctx.enter_context(tc.tile_pool(name="wo_p", bufs=1))
        w1_p = ctx.enter_context(tc.tile_pool(name="w1_p", bufs=1))
        w2_p = ctx.enter_context(tc.tile_pool(name="w2_p", bufs=1))
        qt_p = ctx.enter_context(tc.tile_pool(name="qt_p", bufs=2))
        kt_p = ctx.enter_context(tc.tile_pool(name="kt_p", bufs=2))
        vtok_p = ctx.enter_context(tc.tile_pool(name="vtok_p", bufs=2))
        qhT_p = ctx.enter_context(tc.tile_pool(name="qhT_p", bufs=2))
        kd_p = ctx.enter_context(tc.tile_pool(name="kd_p", bufs=2))
        vh_p = ctx.enter_context(tc.tile_pool(name="vh_p", bufs=2))
        recip_p = ctx.enter_context(tc.tile_pool(name="recip_p", bufs=2))
        attexp_p = ctx.enter_context(tc.tile_pool(name="attexp_p", bufs=4))
        ctxsb_p = ctx.enter_context(tc.tile_pool(name="ctxsb_p", bufs=3))
        ctxt_p = ctx.enter_context(tc.tile_pool(name="ctxt_p", bufs=5))
        ht_p = ctx.enter_context(tc.tile_pool(name="ht_p", bufs=1))
        rt8_p = ctx.enter_context(tc.tile_pool(name="rt8_p", bufs=2))
        lnin_p = ctx.enter_context(tc.tile_pool(name="lnin_p", bufs=2))
        lntmp_p = ctx.enter_context(tc.tile_pool(name="lntmp_p", bufs=5))
        stats_p = ctx.enter_context(tc.tile_pool(name="stats_p", bufs=4))
        ps_p = ctx.enter_context(tc.tile_pool(name="ps_p", bufs=2, space="PSUM"))
        attps_p = ctx.enter_context(tc.tile_pool(name="attps_p", bufs=2, space="PSUM"))
        bcps_p = ctx.enter_context(tc.tile_pool(name="bcps_p", bufs=1, space="PSUM"))
        ctxps_p = ctx.enter_context(tc.tile_pool(name="ctxps_p", bufs=1, space="PSUM"))

        # ---------------- constants ----------------
        ident = consts.tile([128, 128], bf16, tag="ident")
        make_identity(nc, ident)
        ident32 = consts.tile([128, 128], bf16, tag="ident32")
        nc.vector.tensor_scalar(out=ident32, in0=ident, scalar1=32.0,
                                scalar2=None, op0=OP.mult)
        pe_sb = consts.tile([128, 4, D], bf16, tag="pe_sb")
        for sc in range(4):
            pe_st = lnin_p.tile([128, 512], f32, tag="lnin", name=f"pe_st{sc}")
            nc.sync.dma_start(out=pe_st, in_=pe_d[sc * 128:sc * 128 + 128, :])
            nc.vector.tensor_copy(pe_sb[:, sc, :], pe_st)
        bq_sb = consts.tile([128, L, 4], f32, tag="bq_sb")
        nc.sync.dma_start(out=bq_sb, in_=bq_d.rearrange("l (a p) -> p l a", p=128))
        bk_sb = consts.tile([128, L, 4], f32, tag="bk_sb")
        nc.sync.dma_start(out=bk_sb, in_=bk_d.rearrange("l (a p) -> p l a", p=128))
        b1_sb = consts.tile([128, L, 8], f32, tag="b1_sb")
        nc.sync.dma_start(out=b1_sb, in_=b1_d.rearrange("l (a p) -> p l a", p=128))
        ones_r = consts.tile([128, 128], bf16, tag="ones_r")
        nc.vector.memset(ones_r, 1.0)

        def transpose_stream(src, dt=bf16):
            """token-major [128,(16),512] -> new feature-major [128,(4),2048]."""
            dst = streamT.tile([128, 4, T], dt, tag="streamT")
            for dj in range(4):
                for tg in range(NCHUNK // 4):
                    ps = ps_p.tile([128, 512], bf16, tag="ps")
                    for k in range(4):
                        tcn = tg * 4 + k
                        nc.tensor.transpose(
                            ps[:, k * 128:(k + 1) * 128],
                            src[:, tcn, dj * 128:(dj + 1) * 128], ident)
                    nc.vector.tensor_copy(dst[:, dj, tg * 512:(tg + 1) * 512],
                                          ps)
            return dst

        i32 = mybir.dt.int32

        def ln_pre(ps, mvs, tcw):
            """LN stats of PSUM chunk (resid already folded in by an identity
            matmul) + ACT evacuation to bf16; apply happens per slice."""
            st6 = stats_p.tile([128, 6], f32, tag="st6")
            nc.vector.bn_stats(st6, ps)
            nc.vector.bn_aggr(mvs[:, tcw, :], st6)
            tmp = lntmp_p.tile([128, 512], f16, tag="lntmp")
            nc.scalar.activation(tmp, ps, AF.Copy)
            return tmp

        def ln_finalize(mvs):
            """Batched rstd = rsqrt(var+eps) via bit-trick seed + 2 Newton
            iterations (all DVE; avoids ACT Sqrt = table reload vs Exp),
            and nm = -mean*rstd so the apply is (x*rstd + nm)."""
            u = stats_p.tile([128, 4], f32, tag="u4")
            nc.vector.tensor_scalar(out=u, in0=mvs[:, :, 1], scalar1=EPS,
                                    scalar2=None, op0=OP.add)
            y = stats_p.tile([128, 4], f32, tag="y4")
            nc.vector.tensor_scalar(out=y.bitcast(i32), in0=u.bitcast(i32),
                                    scalar1=1, scalar2=-1,
                                    op0=OP.logical_shift_right,
                                    op1=OP.bitwise_xor)
            nc.vector.tensor_scalar(out=y.bitcast(i32), in0=y.bitcast(i32),
                                    scalar1=0x5f3759e0, scalar2=None, op0=OP.add)
            a = stats_p.tile([128, 4], f32, tag="a4")
            for _ in range(2):
                nc.vector.tensor_mul(a, u, y)
                nc.vector.tensor_mul(a, a, y)
                nc.vector.tensor_scalar(out=a, in0=a, scalar1=-0.5, scalar2=1.5,
                                        op0=OP.mult, op1=OP.add)
                nc.vector.tensor_mul(y, y, a)
            nm = stats_p.tile([128, 4], f32, tag="nm4")
            nc.vector.scalar_tensor_tensor(out=nm, in0=mvs[:, :, 0], scalar=-1.0,
                                           in1=y, op0=OP.mult, op1=OP.mult)
            return y, nm

        def ln_apply(tmp, y, nm, tcw, out_ap):
            nc.vector.tensor_scalar(out=out_ap, in0=tmp,
                                    scalar1=y[:, tcw:tcw + 1],
                                    scalar2=nm[:, tcw:tcw + 1],
                                    op0=OP.mult, op1=OP.add)

        # ---------------- prologue: R0 = x + pe ----------------
        # R is THE residual stream, updated in place by LN2 applies (a
        # chunk's last reader is the Wo identity matmul of its own slice)
        R = stream.tile([128, NCHUNK, 512], bf16, tag="resid", bufs=1)
        for tcn in range(NCHUNK):
            xt = lnin_p.tile([128, 512], f32, tag="lnin")
            nc.sync.dma_start(out=xt, in_=x_d[tcn // 4,
                                             (tcn % 4) * 128:(tcn % 4) * 128 + 128, :])
            nc.vector.tensor_add(R[:, tcn, :], xt, pe_sb[:, tcn % 4, :])

        # ---------------- layers ----------------
        for l in range(n_layers):
            # -- weights for this layer --
            wq_t = wq_p.tile([128, 4, D], bf16, tag="wq")
            wk_t = wk_p.tile([128, 4, D], bf16, tag="wk")
            wv_t = wv_p.tile([128, 4, D], f8, tag="wv")
            wo_t = wo_p.tile([128, 4, D], f8, tag="wo")
            w1_t = w1_p.tile([128, 4, FFN], bf16, tag="w1")
            w2_t = w2_p.tile([128, 8, D], bf16, tag="w2")
            for w_t, w_d, nk in ((wq_t, wq_d, 4), (wk_t, wk_d, 4),
                                 (wv_t, wv_d, 4), (wo_t, wo_d, 4),
                                 (w1_t, w1_d, 4), (w2_t, w2_d, 8)):
                nc.sync.dma_start(
                    out=w_t, in_=w_d[l].rearrange("(k p) n -> p k n", p=128))

            rt = transpose_stream(R)  # feature-major stream
            A = stream.tile([128, NCHUNK, 512], bf16, tag="stream", bufs=1)
            at = streamT.tile([128, 4, T], bf16, tag="streamT")
            last = l == n_layers - 1
            ffn_state = {}

            def ffn_chunk(fs, step, at=at, A=A, w1_t=w1_t, w2_t=w2_t,
                          l=l, last=last, ffn_state=ffn_state):
                """FFN for slice fs, emitted interleaved into the next
                slice's attention block loop (layer state bound at def time
                so the carried-over closure uses ITS layer's tiles).
                steps 0-3: W1 fc-pair + relu; steps 4-7: W2 token-chunk +
                ln_pre (finalize+apply at 7)."""
                t0f = fs * 512
                if step == 0:
                    ffn_state["ht"] = ht_p.tile([128, 8, 512], bf16, tag="ht", name=f"ht{fs}")
                    ffn_state["mvs"] = stats_p.tile([128, 4, 2], f32, tag="mvs", name=f"mvsf{fs}")
                    ffn_state["tmps"] = []
                ht_sl = ffn_state["ht"]
                if step < 4:
                    for fc in (2 * step, 2 * step + 1):
                        ps = ps_p.tile([128, 512], f32, tag="ps")
                        for dk in range(4):
                            nc.tensor.matmul(ps, w1_t[:, dk, fc * 128:fc * 128 + 128],
                                             at[:, dk, t0f:t0f + 512],
                                             start=(dk == 0), stop=(dk == 3))
                        nc.scalar.activation(ht_sl[:, fc, :], ps, AF.Relu,
                                             bias=b1_sb[:, l, fc:fc + 1].opt())
                    return
                tcw = step - 4
                tcn = fs * 4 + tcw
                ps = ps_p.tile([128, 512], f32, tag="ps")
                nc.tensor.matmul(ps, ident, A[:, tcn, :], start=True, stop=False)
                for fk in range(8):
                    nc.tensor.matmul(ps, ht_sl[:, fk, tcw * 128:tcw * 128 + 128],
                                     w2_t[:, fk, :], start=False, stop=(fk == 7))
                ffn_state["tmps"].append(ln_pre(ps, ffn_state["mvs"], tcw))
                if step == 7:
                    y2, nm2 = ln_finalize(ffn_state["mvs"])
                    for tc2 in range(4):
                        tcn2 = fs * 4 + tc2
                        if last:
                            ot = lnin_p.tile([128, 512], f32, tag="lnin")
                            ln_apply(ffn_state["tmps"][tc2], y2, nm2, tc2, ot)
                            nc.sync.dma_start(
                                out=ov[tcn2 // 4,
                                       (tcn2 % 4) * 128:(tcn2 % 4) * 128 + 128, :],
                                in_=ot)
                        else:
                            ln_apply(ffn_state["tmps"][tc2], y2, nm2, tc2,
                                     R[:, tcn2, :])


            def emit_qkv(ts):
                """Q/K/V projections + rearrange DMAs for one slice.

                Layouts (i = dg*64 + sm "dg-major" row order everywhere):
                  qd [64e, blk, 512 j]  j = dg*64+sm (att columns = q-rows)
                  kd [64e, mc, blk, 128]  lhsT panel for chunk mc covers
                     k-rows i in [128mc, 128mc+128) as (par, sm)
                  vh [128p, blk, mc, 65]  p = par*64+sm' -> k-row i=128mc+p,
                     free = 64 V feats of group dg'=2mc+par + ones column
                """
                t0 = ts * 512
                qt_t = qt_p.tile([128, 4, 512], bf16, tag="qt", name=f"qt{ts}")
                kt_t = kt_p.tile([128, 4, 512], bf16, tag="kt", name=f"kt{ts}")
                for (w_t, b_sb, dst) in ((wq_t, bq_sb, qt_t), (wk_t, bk_sb, kt_t)):
                    for dc in range(4):
                        ps = ps_p.tile([128, 512], f32, tag="ps", name=f"ps{ts}{dc}")
                        for dk in range(4):
                            nc.tensor.matmul(ps, w_t[:, dk, dc * 128:dc * 128 + 128],
                                             rt[:, dk, t0:t0 + 512],
                                             start=(dk == 0), stop=(dk == 3))
                        nc.vector.tensor_scalar(
                            out=dst[:, dc, :], in0=ps,
                            scalar1=b_sb[:, l, dc:dc + 1].opt(), scalar2=None,
                            op0=OP.add)
                kd_t = kd_p.tile([64, 4, 8, 128], bf16, tag="kd", name=f"kd{ts}")
                kd_v = kd_t.rearrange("p m b (r s) -> p m b r s", r=2)
                kt_v = kt_t.rearrange("p m (b s) -> p m b s", b=8)
                nc.sync.dma_start(out=kd_v[0:64, :, :, 0, :],
                                  in_=kt_v[0:64, :, :, :])
                nc.sync.dma_start(out=kd_v[0:64, :, :, 1, :],
                                  in_=kt_v[64:128, :, :, :])
                qd_sl = qhT_p.tile([64, 8, 512], bf16, tag="qhT", name=f"qd{ts}")
                qd_v = qd_sl.rearrange("p b (a c) -> p b a c", a=4)
                qt_v = qt_t.rearrange("p a (b c) -> p b a c", b=8)
                for dt4 in range(4):
                    nc.sync.dma_start(out=qd_v[0:64, :, dt4, 0:64],
                                      in_=qt_v[0:64, :, dt4, :])
                    nc.sync.dma_start(out=qd_v[0:64, :, dt4, 64:128],
                                      in_=qt_v[64:128, :, dt4, :])
                rt8_sl = rt8_p.tile([128, 4, 512], f8, tag="rt8", name=f"rt8{ts}")
                for dk in range(4):
                    nc.scalar.copy(rt8_sl[:, dk, :], rt[:, dk, t0:t0 + 512])
                vtok_sl = vtok_p.tile([128, 4, 512], bf16, tag="vtok", name=f"vt{ts}")
                for tcw in range(4):
                    ps = ps_p.tile([128, 512], f32, tag="ps", name=f"psv{ts}{tcw}")
                    for st2 in range(2):
                        nc.tensor.matmul(
                            ps,
                            rt8_sl[:, 2 * st2:2 * st2 + 2, tcw * 128:tcw * 128 + 128],
                            wv_t[:, 2 * st2:2 * st2 + 2, :],
                            start=(st2 == 0), stop=(st2 == 1), perf_mode=PM.DoubleRow)
                    nc.vector.tensor_scalar(out=vtok_sl[:, tcw, :], in0=ps,
                                            scalar1=1.0 / 32.0, scalar2=None,
                                            op0=OP.mult)
                vh_t = vh_p.tile([128, 8, 4, 65], bf16, tag="vh", name=f"vh{ts}")
                nc.vector.memset(vh_t[:, :, :, 64:65], 1.0)
                for mc in range(4):
                    for par in range(2):
                        f0 = (2 * mc + par) * 64
                        nc.sync.dma_start(
                            out=vh_t[par * 64:par * 64 + 64, 0::2, mc, 0:64],
                            in_=vtok_sl[0:64, :, f0:f0 + 64])
                        nc.sync.dma_start(
                            out=vh_t[par * 64:par * 64 + 64, 1::2, mc, 0:64],
                            in_=vtok_sl[64:128, :, f0:f0 + 64])
                return qd_sl, kd_t, vh_t

            slice_ops = emit_qkv(0)
            for ts in range(NSLICE):
                qd_sl, kd_t, vh_t = slice_ops

                # -- attention: 8 blocks of 64 tokens, software-pipelined
                # (ctx of block i-1 between logits of block i), with the
                # previous slice's FFN chunks interleaved so the PE-heavy
                # FFN overlaps the ACT-heavy softmax --
                ctx_ch = []
                axs_prev, prev_blk, ctxc = None, None, None
                for step in range(9):
                    blk = step if step < 8 else None
                    axs = []
                    if blk is not None:
                        for mp in range(2):
                            aps = attps_p.tile([128, 1024], f32, tag="attps")
                            for mh in range(2):
                                nc.tensor.matmul(
                                    aps[:, mh * 512:mh * 512 + 512],
                                    kd_t[0:64, 2 * mp + mh, blk, :],
                                    qd_sl[0:64, blk, :], start=True, stop=True)
                            ax = attexp_p.tile([128, 1024], bf16, tag="attexp")
                            nc.scalar.activation(ax, aps, AF.Exp,
                                                 scale=float(DH ** -0.5))
                            axs.append(ax)
                    if prev_blk is not None:
                        pb = prev_blk
                        half = pb % 2
                        cps = ctxps_p.tile([65, 512], f32, tag="ctxps")
                        for mc in range(4):
                            nc.tensor.matmul(
                                cps, vh_t[:, pb, mc, :],
                                axs_prev[mc // 2][:, (mc % 2) * 512:(mc % 2) * 512 + 512],
                                start=(mc == 0), stop=(mc == 3))
                        csb = ctxsb_p.tile([65, 512], bf16, tag="ctxsb")
                        nc.vector.tensor_copy(csb, cps)
                        bcp = bcps_p.tile([64, 512], f32, tag="bcp")
                        nc.tensor.matmul(bcp, ones_r[64:65, 0:64],
                                         csb[64:65, :], start=True, stop=True)
                        rcf = recip_p.tile([64, 512], f32, tag="recip")
                        nc.vector.reciprocal_approx_fast(out=rcf, in_=bcp)
                        nc.vector.tensor_mul(csb[0:64, :], csb[0:64, :], rcf)
                        csb_v = csb.rearrange("p (a c) -> p a c", a=4)
                        if half == 0:
                            ctxc = ctxt_p.tile([128, 4, 128], bf16, tag="ctxt")
                        c0 = half * 64
                        nc.sync.dma_start(out=ctxc[0:64, :, c0:c0 + 64],
                                          in_=csb_v[0:64, :, 0:64])
                        nc.sync.dma_start(out=ctxc[64:128, :, c0:c0 + 64],
                                          in_=csb_v[0:64, :, 64:128])
                        if half == 1:
                            ctxc8 = ctxt_p.tile([128, 4, 128], f8, tag="ctxt8")
                            nc.scalar.copy(ctxc8, ctxc)
                            ctx_ch.append(ctxc8)
                    axs_prev, prev_blk = axs, blk

                    if ts > 0 and step < 8:
                        ffn_chunk(ts - 1, step)
                    if step == 3 and ts + 1 < NSLICE:
                        slice_ops = emit_qkv(ts + 1)

                # -- Wo projection + residual (identity matmul) + LN1 --
                mvs1 = stats_p.tile([128, 4, 2], f32, tag="mvs")
                tmps1 = []
                for tcw in range(4):
                    tcn = ts * 4 + tcw
                    ps = ps_p.tile([128, 512], f32, tag="ps")
                    nc.tensor.matmul(ps, ident32, R[:, tcn, :],
                                     start=True, stop=False)
                    for st2 in range(2):
                        nc.tensor.matmul(
                            ps, ctx_ch[tcw][:, 2 * st2:2 * st2 + 2, :],
                            wo_t[:, 2 * st2:2 * st2 + 2, :],
                            start=False, stop=(st2 == 1), perf_mode=PM.DoubleRow)
                    tmps1.append(ln_pre(ps, mvs1, tcw))
                y1, nm1 = ln_finalize(mvs1)
                for tcw in range(4):
                    ln_apply(tmps1[tcw], y1, nm1, tcw, A[:, ts * 4 + tcw, :])

                # -- feature-major transpose of this slice of A --
                for dj in range(4):
                    ps = ps_p.tile([128, 512], bf16, tag="ps")
                    for k in range(4):
                        tcn = ts * 4 + k
                        nc.tensor.transpose(
                            ps[:, k * 128:(k + 1) * 128],
                            A[:, tcn, dj * 128:(dj + 1) * 128], ident)
                    nc.vector.tensor_copy(at[:, dj, ts * 512:(ts + 1) * 512],
                                          ps)

            # trailing FFN for the last slice
            for step in range(8):
                ffn_chunk(NSLICE - 1, step)


# ---------------------------------------------------------------------------
# host side
# ---------------------------------------------------------------------------

def _numpy_reference(x, pe, Wq, bq, Wk, bk, Wv, bv, Wo, bo, ln1_g, ln1_b,
                     W1, b1, W2, b2, ln2_g, ln2_b):
    """Exact fp64->fp32 fallback, mirrors reference.py (used only if the
    fast-path constant assumptions do not hold)."""
    def ln(x_, g, b_):
        mu = x_.mean(-1, keepdims=True)
        var = ((x_ - mu) ** 2).mean(-1, keepdims=True)
        return (x_ - mu) / np.sqrt(var + EPS) * g + b_
    out = x.astype(np.float64) + pe.astype(np.float64)
    scale = DH ** -0.5
    for l in range(L):
        Q = out @ Wq[l].astype(np.float64) + bq[l]
        K = out @ Wk[l].astype(np.float64) + bk[l]
        V = out @ Wv[l].astype(np.float64) + bv[l]
        Qh = Q.reshape(B * H, S, DH)
        Kh = K.reshape(B * H, S, DH)
        Vh = V.reshape(B * H, S, DH)
        att = np.einsum("bqd,bkd->bqk", Qh, Kh) * scale
        att = att - att.max(-1, keepdims=True)
        att = np.exp(att)
        att /= att.sum(-1, keepdims=True)
        ctxv = np.einsum("bqk,bkd->bqd", att, Vh).reshape(B, S, D)
        a = ln(ctxv @ Wo[l].astype(np.float64) + bo[l] + out, ln1_g[l], ln1_b[l])
        h = np.maximum(a @ W1[l].astype(np.float64) + b1[l], 0.0)
        out = ln(h @ W2[l].astype(np.float64) + b2[l] + a, ln2_g[l], ln2_b[l])
    return out.reshape(B, S * D).astype(np.float32)


def _fast_path_ok(inputs):
    z = lambda a: np.all(np.asarray(a) == 0.0)
    o = lambda a: np.all(np.asarray(a) == 1.0)
    return (z(inputs["bv"]) and z(inputs["bo"]) and z(inputs["b2"])
            and o(inputs["ln1_g"]) and z(inputs["ln1_b"])
            and o(inputs["ln2_g"]) and z(inputs["ln2_b"]))


def kernel(**inputs):
    inputs = {k: np.asarray(v) for k, v in inputs.items()}
    if not _fast_path_ok(inputs):
        return _numpy_reference(**inputs)

    res = _run(inputs)
    return np.concatenate([res.results[i]["out"] for i in range(NCORES)], axis=0)


def _run(inputs, trace=False, **kw):
    from concourse.bass_utils import run_bass_kernel_spmd

    if "prog" not in _PROG_CACHE:
        _PROG_CACHE["prog"] = _build_program(L)
    nc = _PROG_CACHE["prog"]

    bf = ml_dtypes.bfloat16
    f8 = ml_dtypes.float8_e4m3
    shared = {
        "pe": inputs["pe"].astype(np.float32),
        "wq": inputs["Wq"].astype(bf), "wk": inputs["Wk"].astype(bf),
        "wv": (inputs["Wv"] * 32.0).astype(f8),
        "wo": (inputs["Wo"] * 32.0).astype(f8),
        "w1": inputs["W1"].astype(bf), "w2": inputs["W2"].astype(bf),
        "bq": inputs["bq"].astype(np.float32),
        "bk": inputs["bk"].astype(np.float32),
        "b1": inputs["b1"].astype(np.float32),
    }
    x = inputs["x"].astype(np.float32)
    in_maps = [dict(shared, x=np.ascontiguousarray(x[i * BL:(i + 1) * BL]))
               for i in range(NCORES)]
    return run_bass_kernel_spmd(nc, in_maps, list(range(NCORES)),
                                trace=trace, **kw)


if __name__ == "__main__":
    import reference
    ins = {k: np.asarray(v) for k, v in reference.setup_inputs().items()}
    got = kernel(**ins)
    print("out shape:", got.shape, got.dtype)

